# revision 10
# baseline (speedup 1.0000x reference)
"""Fused cross-attention kernel for Trainium2, data-parallel over batch on 8 cores.

Per core (one batch element):
  tn   = LayerNorm(text)                      (gamma folded into Wk/Wv on host)
  Q^T  = Wq^T @ X^T   (X^T pre-transposed on host, bf16)
  K^T  = Wk^T @ tn^T,  V = tn @ Wv            (tn^T via PE transpose)
  per q-tile of 128 rows, per head:
    S    = Q_h^T.T @ K_h^T                    (q on partitions, l on free)
    E    = exp(S * scale)                     (no max-sub: |S*scale| < 2)
    A    = (E * mask) / sum(E * mask)         (masked softmax)
    C^T  = V_h.T'd via lhsT=V_h, rhs=A^T      (A^T via PE transpose)
  out  = concat_h(C) @ Wo                     (natural layout, DMA out)
"""

import sys

sys.path.insert(0, "/opt/trn_rl_repo")

import numpy as np
import ml_dtypes

import concourse.bass as bass
import concourse.mybir as mybir
import concourse.tile as tile
from concourse import bacc
from concourse.bass_utils import run_bass_kernel_spmd
from concourse.masks import make_identity

N_CORES = 8
B, T, S_, D, L, H = 8, 64, 196, 512, 77, 4
DH = D // H  # 128
NQ = T * S_  # 12544
LN_EPS = 1e-6
SCALE = float(DH) ** -0.5
P = 128
NCH = D // P  # 4 chunks of the feature dim

F32 = mybir.dt.float32
BF16 = mybir.dt.bfloat16

LAST_RESULTS = None  # BassKernelResults of the most recent run (for test harness)
_PROGRAM_CACHE = {}


def build_program(nq=NQ, stage=99):
    """One SPMD program; all 8 cores run it on their own batch element."""
    nc = bacc.Bacc("TRN2", target_bir_lowering=False, debug=False, num_devices=N_CORES)

    xt = nc.dram_tensor("xt", [D, nq], BF16, kind="ExternalInput").ap()
    text = nc.dram_tensor("text", [P, D], F32, kind="ExternalInput").ap()
    maskv = nc.dram_tensor("maskv", [P, 1], F32, kind="ExternalInput").ap()
    maskb = nc.dram_tensor("maskb", [P], BF16, kind="ExternalInput").ap()
    wq = nc.dram_tensor("wq", [D, D], BF16, kind="ExternalInput").ap()
    wk = nc.dram_tensor("wk", [D, D], BF16, kind="ExternalInput").ap()
    wv = nc.dram_tensor("wv", [D, D], BF16, kind="ExternalInput").ap()
    wo = nc.dram_tensor("wo", [D, D], BF16, kind="ExternalInput").ap()
    out = nc.dram_tensor("out", [nq, D], F32, kind="ExternalOutput").ap()

    ntiles = nq // P
    # q-tile groups of up to 4 (512 q rows per Q-projection pass)
    groups = []
    t0 = 0
    while t0 < ntiles:
        gt = min(4, ntiles - t0)
        groups.append((t0, gt))
        t0 += gt

    with tile.TileContext(nc) as tc:
        with (
            tc.tile_pool(name="const", bufs=1) as const,
            tc.tile_pool(name="xtp", bufs=2) as xtp,
            tc.tile_pool(name="qtp", bufs=2) as qtp,
            tc.tile_pool(name="attp", bufs=3) as attp,
            tc.tile_pool(name="smalls", bufs=24) as smalls,
            tc.tile_pool(name="outp", bufs=3) as outp,
            tc.tile_pool(name="ps_qt", bufs=2, space="PSUM") as ps_qt,
            tc.tile_pool(name="ps_sc", bufs=2, space="PSUM") as ps_sc,
            tc.tile_pool(name="ps_at", bufs=2, space="PSUM") as ps_at,
            tc.tile_pool(name="ps_ctx", bufs=1, space="PSUM") as ps_ctx,
            tc.tile_pool(name="ps_out", bufs=1, space="PSUM") as ps_out,
        ):
            # ---- constants / prolog ----
            ident = const.tile([P, P], BF16)
            make_identity(nc, ident)

            wq_sb = const.tile([P, NCH, D], BF16, tag="wq")
            wk_sb = const.tile([P, NCH, D], BF16, tag="wk")
            wv_sb = const.tile([P, NCH, D], BF16, tag="wv")
            wo_sb = const.tile([P, NCH, D], BF16, tag="wo")
            for w_sb, w_dram in ((wq_sb, wq), (wk_sb, wk), (wv_sb, wv), (wo_sb, wo)):
                nc.sync.dma_start(
                    out=w_sb[:], in_=w_dram.rearrange("(c p) n -> p c n", p=P)
                )

            text_sb = const.tile([P, D], F32, tag="text")
            nc.sync.dma_start(out=text_sb[:], in_=text)
            maskv_sb = const.tile([P, 1], F32, tag="maskv")
            nc.sync.dma_start(out=maskv_sb[:], in_=maskv)
            maskb_sb = const.tile([P, P], BF16, tag="maskb")
            maskb_bcast = bass.AP(
                tensor=maskb.tensor, offset=maskb.offset, ap=[[0, P]] + maskb.ap
            )
            nc.gpsimd.dma_start(out=maskb_sb[:], in_=maskb_bcast)

            # ---- LayerNorm of text (77 valid rows; pad rows are zeros) ----
            stats = smalls.tile([P, 6], F32, tag="bnstats")
            nc.vector.bn_stats(out=stats[:], in_=text_sb[:])
            mv = smalls.tile([P, 2], F32, tag="bnaggr")
            nc.vector.bn_aggr(out=mv[:], in_=stats[:])
            eps_sb = smalls.tile([P, 1], F32, tag="eps")
            nc.vector.memset(eps_sb[:], LN_EPS)
            std = smalls.tile([P, 1], F32, tag="std")
            nc.scalar.activation(
                std[:], mv[:, 1:2], mybir.ActivationFunctionType.Sqrt, bias=eps_sb[:]
            )
            rstd = smalls.tile([P, 1], F32, tag="rstd")
            nc.vector.reciprocal(rstd[:], std[:])
            tn_sb = const.tile([P, D], BF16, tag="tn")
            nc.vector.tensor_scalar(
                out=tn_sb[:],
                in0=text_sb[:],
                scalar1=mv[:, 0:1],
                scalar2=rstd[:],
                op0=mybir.AluOpType.subtract,
                op1=mybir.AluOpType.mult,
            )

            # ---- tn^T (D on partitions), K^T, V ----
            tnT_sb = const.tile([P, NCH, P], BF16, tag="tnT")
            for c in range(NCH):
                ps_t = ps_at.tile([P, P], BF16, tag="ps_trb")
                nc.tensor.transpose(ps_t[:], tn_sb[:, c * P : (c + 1) * P], ident[:])
                nc.scalar.copy(tnT_sb[:, c, :], ps_t[:])

            kt_sb = const.tile([P, H, P], BF16, tag="kt")
            for dch in range(NCH):
                ps_k = ps_sc.tile([P, P], F32, tag="ps_s")
                for kc in range(NCH):
                    nc.tensor.matmul(
                        ps_k[:],
                        wk_sb[:, kc, dch * P : (dch + 1) * P],
                        tnT_sb[:, kc, :],
                        start=(kc == 0),
                        stop=(kc == NCH - 1),
                    )
                nc.scalar.copy(kt_sb[:, dch, :], ps_k[:])

            ps_v = ps_qt.tile([P, D], F32, tag="ps_q")
            for kc in range(NCH):
                nc.tensor.matmul(
                    ps_v[:],
                    tnT_sb[:, kc, :],
                    wv_sb[:, kc, :],
                    start=(kc == 0),
                    stop=(kc == NCH - 1),
                )
            v_sb = const.tile([P, D], BF16, tag="v")
            nc.vector.tensor_scalar_mul(v_sb[:], ps_v[:], maskv_sb[:])

            # ---- main loop over q-tile groups ----
            for t0, gt in groups:
                qg = gt * P
                q0 = t0 * P

                xt_sb = xtp.tile([P, NCH, qg], BF16, tag="xt")
                nc.sync.dma_start(
                    out=xt_sb[:],
                    in_=xt.rearrange("(c p) q -> p c q", p=P)[:, :, q0 : q0 + qg],
                )

                qt_sb = qtp.tile([P, H, qg], BF16, tag="qt")
                for dch in range(NCH):
                    ps_q = ps_qt.tile([P, qg], F32, tag="ps_q")
                    for kc in range(NCH):
                        nc.tensor.matmul(
                            ps_q[:],
                            wq_sb[:, kc, dch * P : (dch + 1) * P],
                            xt_sb[:, kc, :],
                            start=(kc == 0),
                            stop=(kc == NCH - 1),
                        )
                    nc.scalar.copy(qt_sb[:, dch, :], ps_q[:])

                for t in range(gt):
                    if stage < 2:
                        continue
                    tq = slice(t * P, (t + 1) * P)
                    ps_c = ps_ctx.tile([P, D], F32, tag="ps_c")
                    ctxT_sb = attp.tile([P, H, P], BF16, tag="ctxT")
                    for h in range(H):
                        ps_s = ps_sc.tile([P, P], F32, tag="ps_s")
                        nc.tensor.matmul(
                            ps_s[:], qt_sb[:, h, tq], kt_sb[:, h, :],
                            start=True, stop=True,
                        )
                        if stage < 3:
                            continue
                        exp_sb = attp.tile([P, P], BF16, tag="exp")
                        nc.scalar.activation(
                            exp_sb[:], ps_s[:], mybir.ActivationFunctionType.Exp,
                            scale=SCALE,
                        )
                        if stage < 4:
                            continue
                        me_sb = attp.tile([P, P], BF16, tag="me")
                        nc.vector.tensor_mul(me_sb[:], exp_sb[:], maskb_sb[:])
                        sumexp = smalls.tile([P, 1], F32, tag="sumexp")
                        nc.vector.reduce_sum(
                            out=sumexp[:], in_=me_sb[:], axis=mybir.AxisListType.X
                        )
                        if stage < 41:
                            continue
                        recip = smalls.tile([P, 1], F32, tag="recip")
                        nc.vector.reciprocal(recip[:], sumexp[:])
                        if stage < 42:
                            continue
                        attn_sb = attp.tile([P, P], BF16, tag="attn")
                        nc.vector.tensor_scalar_mul(attn_sb[:], me_sb[:], recip[:])
                        if stage < 50:
                            continue
                        ps_a = ps_at.tile([P, P], BF16, tag="ps_trb")
                        nc.tensor.transpose(ps_a[:], attn_sb[:], ident[:])
                        attnT_sb = attp.tile([P, P], BF16, tag="attnT")
                        nc.vector.tensor_copy(attnT_sb[:], ps_a[:])
                        if stage < 60:
                            continue
                        nc.tensor.matmul(
                            ps_c[:, h * P : (h + 1) * P],
                            v_sb[:, h * P : (h + 1) * P],
                            attnT_sb[:],
                            start=True, stop=True,
                        )
                    if stage < 60:
                        continue
                    nc.vector.tensor_copy(ctxT_sb[:], ps_c[:].rearrange("p (c n) -> p c n", c=H))

                    if stage < 70:
                        continue
                    ps_o = ps_out.tile([P, D], F32, tag="ps_o")
                    for h in range(H):
                        nc.tensor.matmul(
                            ps_o[:],
                            ctxT_sb[:, h, :],
                            wo_sb[:, h, :],
                            start=(h == 0),
                            stop=(h == H - 1),
                        )
                    out_sb = outp.tile([P, D], F32, tag="out")
                    nc.scalar.copy(out_sb[:], ps_o[:])
                    nc.sync.dma_start(
                        out=out[q0 + t * P : q0 + (t + 1) * P, :], in_=out_sb[:]
                    )

    nc.compile()
    return nc


def _get_program(nq=NQ):
    if nq not in _PROGRAM_CACHE:
        _PROGRAM_CACHE[nq] = build_program(nq)
    return _PROGRAM_CACHE[nq]


def prep_core_inputs(visual_feat, text_feat, token_mask, wq, wk, wv, wo,
                     ln_gamma, ln_beta):
    """Host-side prep: shard over batch, fold gamma, transpose X, cast bf16."""
    vf = np.ascontiguousarray(visual_feat.reshape(B, -1, D))
    wk2 = (ln_gamma[:, None] * wk).astype(np.float32)
    wv2 = (ln_gamma[:, None] * wv).astype(np.float32)
    wq_b = wq.astype(ml_dtypes.bfloat16)
    wk_b = wk2.astype(ml_dtypes.bfloat16)
    wv_b = wv2.astype(ml_dtypes.bfloat16)
    wo_b = wo.astype(ml_dtypes.bfloat16)

    in_maps = []
    for b in range(B):
        xt = np.ascontiguousarray(vf[b].T).astype(ml_dtypes.bfloat16)
        text = np.zeros((P, D), np.float32)
        text[:L] = text_feat[b]
        m = token_mask[b].astype(np.float32)
        maskv = np.zeros((P, 1), np.float32)
        maskv[:L, 0] = m
        maskb = np.zeros((P,), ml_dtypes.bfloat16)
        maskb[:L] = m.astype(ml_dtypes.bfloat16)
        in_maps.append({
            "xt": xt, "text": text, "maskv": maskv, "maskb": maskb,
            "wq": wq_b, "wk": wk_b, "wv": wv_b, "wo": wo_b,
        })
    # LN beta correction: beta affects scores only via a softmax-invariant
    # per-row constant, and the output via a constant row added everywhere.
    out_corr = (ln_beta.astype(np.float64) @ wv2.astype(np.float64)
                @ wo.astype(np.float64)).astype(np.float32)
    return in_maps, out_corr


def kernel(visual_feat, text_feat, token_mask, Wq, Wk, Wv, Wo, ln_gamma, ln_beta):
    global LAST_RESULTS
    visual_feat = np.asarray(visual_feat, np.float32)
    text_feat = np.asarray(text_feat, np.float32)
    token_mask = np.asarray(token_mask)

    in_maps, out_corr = prep_core_inputs(
        visual_feat, text_feat, token_mask,
        np.asarray(Wq, np.float32), np.asarray(Wk, np.float32),
        np.asarray(Wv, np.float32), np.asarray(Wo, np.float32),
        np.asarray(ln_gamma, np.float32), np.asarray(ln_beta, np.float32),
    )
    nc = _get_program()
    res = run_bass_kernel_spmd(nc, in_maps, core_ids=list(range(N_CORES)))
    LAST_RESULTS = res
    out = np.stack([res.results[b]["out"] for b in range(B)], axis=0)
    if np.any(out_corr):
        out = out + out_corr[None, None, :]
    return out.reshape(B, T, S_, D)
